# revision 22
# baseline (speedup 1.0000x reference)
"""Fused causal-attention block (QKV proj + causal softmax attention + out proj
+ residual + LayerNorm) on 8 Trainium2 NeuronCores.

Sharding: core c -> batch b = c//4, head-group r = c%4 (heads 4r..4r+3,
d' columns 256r..256r+256).  Single pipelined stream per core:

  for n-chunk nr (512 rows):   K/Q/V projections for the head group
  interleaved per q-tile qt:   flash-style causal attention (no max
                               subtraction -- scores are O(1)),
  then per q-tile:             partial output projection
                               P_qt = ctx_qt @ Wo[:, cols]^T  [512, 1024]
                               ReduceScatter(add) over the batch's 4 cores
                               -> this core's 128 rows of q-tile qt,
  then per received chunk:     + residual, LayerNorm, store.

QKV chunks, attention pairs, partial projections and collectives are
interleaved in emission order so the PE never drains between phases; the
trailing exposed work after the last attention tile is just one partial
projection + ReduceScatter + LayerNorm on 128 rows.

Core (b, r) returns out rows {512*qt + 128*r .. +128} for qt in 0..3 (the
ReduceScatter hands rank r the r-th 128-row slice of each 512-row q-tile);
the host reassembles.

All matmuls run as float32r (full-rate fp32 on the PE).  The causal mask on
diagonal 128x128 blocks is applied by multiplying the exponentiated scores
with an upper-triangular 0/1 matrix.  Softmax denominators come from an
all-ones column appended to V; their reciprocals are computed with the 512
denominators spread over 64 partitions via SBUF->SBUF DMA (partition-parallel
reciprocal), then broadcast with a ones-column matmul.  The two heads of a
partition-tile pair compute their scores back-to-back at PE base partitions
0/64 into one shared [128, 2, 512] PSUM tile that a single strided ACT call
exponentiates for both heads.
"""

import numpy as np
F16 = np.float16
B, N, D = 2, 2048, 1024
H, DH = 16, 64
NCORES = 8
HPC = 4          # heads per core
DP = HPC * DH    # 256 d' columns per core
NQ = N // 4      # 512 rows per q-tile
LN_EPS = 1e-5
GROUPS = [[0, 1, 2, 3], [4, 5, 6, 7]]

_CACHE = {}


def _build(flags):
    """Build+compile the Bacc program. flags = (has_qkv_bias, has_gamma, has_beta)."""
    import concourse.bass as bass
    import concourse.bacc as bacc
    import concourse.tile as tile
    from concourse import mybir
    from contextlib import ExitStack

    has_qkv_bias, has_gamma, has_beta = flags
    f32 = mybir.dt.float32
    f32r = mybir.dt.float32r
    f16 = mybir.dt.float16
    bf16 = mybir.dt.bfloat16
    AF = mybir.ActivationFunctionType
    ALU = mybir.AluOpType

    nc = bacc.Bacc(
        trn_type="TRN2",
        target_bir_lowering=False,
        debug=False,
        num_devices=NCORES,
    )

    xT = nc.dram_tensor("xT", [128, 4, 8, 512], f16, kind="ExternalInput").ap()
    xres = nc.dram_tensor("xres", [NQ, D], f32, kind="ExternalInput").ap()
    wqT = nc.dram_tensor("wqT", [128, 8, DP], f16, kind="ExternalInput").ap()
    wkT = nc.dram_tensor("wkT", [128, 8, DP], f16, kind="ExternalInput").ap()
    wvT = nc.dram_tensor("wvT", [128, 8, DP], f16, kind="ExternalInput").ap()
    woT = nc.dram_tensor("woT", [128, 2, D], f16, kind="ExternalInput").ap()
    out = nc.dram_tensor("out", [NQ, D], f32, kind="ExternalOutput").ap()
    if has_qkv_bias:
        bqkv = nc.dram_tensor("bqkv", [1, 3, DP], f32r, kind="ExternalInput").ap()
    if has_gamma:
        gamma_d = nc.dram_tensor("gamma", [D], f32, kind="ExternalInput").ap()
    if has_beta:
        beta_d = nc.dram_tensor("beta", [D], f32, kind="ExternalInput").ap()

    # multiplicative causal mask for diagonal blocks: keep k <= q
    # (partition p = k offset, free c = q offset)
    tri_np = np.triu(np.ones((128, 128), np.float32))
    tri_d = nc.inline_tensor(np.ascontiguousarray(tri_np.astype(np.float16)),
                             name="tri_const").ap()

    with tile.TileContext(nc) as tc, ExitStack() as ctx, \
            nc.allow_low_precision(reason="float32r carries full fp32 bits"):
        singles = ctx.enter_context(tc.tile_pool(name="singles", bufs=1))
        qkv_pool = ctx.enter_context(tc.tile_pool(name="qkv", bufs=1))

        # weights, striped k-on-partitions
        wq_sb = singles.tile([128, 8, DP], f16, tag="wq")
        wk_sb = singles.tile([128, 8, DP], f16, tag="wk")
        wv_sb = singles.tile([128, 8, DP], f16, tag="wv")
        wo_sb = singles.tile([128, 2, D], f16, tag="wo")

        tri_sb = singles.tile([128, 128], f16, tag="tri")
        ones_f32 = singles.tile([128, 64], f32, tag="ones_f32")
        nc.vector.memset(ones_f32, 1.0)
        ones64 = singles.tile([1, 64], f16, tag="ones64")
        nc.vector.tensor_copy(out=ones64, in_=ones_f32[0:1, :])
        eps_sb = singles.tile([128, 1], f32, tag="eps")
        nc.vector.memset(eps_sb, LN_EPS)
        if has_qkv_bias:
            o512f = singles.tile([1, 512], f32, tag="o512f")
            nc.vector.memset(o512f, 1.0)
            ones512 = singles.tile([1, 512], f32r, tag="ones512")
            nc.vector.tensor_copy(out=ones512, in_=o512f)
            bqkv_sb = singles.tile([1, 3, DP], f32r, tag="bqkv")
        if has_gamma:
            gamma_sb = singles.tile([128, D], f32, tag="gamma")
        if has_beta:
            beta_sb = singles.tile([128, D], f32, tag="beta")

        # persistent activations
        qT_sb = qkv_pool.tile([128, 2, N], f16, tag="qT")   # Q^T [d'(256), n]
        kT_sb = qkv_pool.tile([128, 2, N], f16, tag="kT")   # K^T [d'(256), n]
        v_sb = qkv_pool.tile([128, 16, HPC, DH + 1], f16, tag="v")  # V + ones
        ctx_sb = qkv_pool.tile([128, 2, N], f16, tag="ctxT")  # normalized ctx^T
        nc.vector.tensor_copy(
            out=v_sb[:, :, :, DH:DH + 1],
            in_=ones_f32.rearrange("p (a b c) -> p a b c", a=16, b=4))


        # ---- input DMAs (host-prearranged contiguous layouts) ----
        xt_pool = ctx.enter_context(tc.tile_pool(name="xt", bufs=4))
        xc = []
        for nr in range(4):
            xc.append(xt_pool.tile([128, 8, 512], f16, tag="xc",
                                   name=f"xc{nr}"))
        # first chunk: per-ko pieces round-robin over three queues so the
        # first K accumulation chain can start as pieces land; later chunks
        # as halves on sync/gpsimd.
        nc.sync.dma_start(wk_sb, wkT)
        nc.scalar.dma_start(tri_sb, tri_d)
        dq = [nc.sync, nc.gpsimd, nc.scalar]
        for ko in range(8):
            dq[ko % 3].dma_start(xc[0][:, ko, :], xT[:, 0, ko, :])
        nc.scalar.dma_start(wq_sb, wqT)
        nc.gpsimd.dma_start(wv_sb, wvT)
        dq2 = [nc.sync, nc.gpsimd, nc.scalar]
        for nr in range(1, 4):
            dq2[(2 * nr) % 3].dma_start(xc[nr][:, 0:4, :], xT[:, nr, 0:4, :])
            dq2[(2 * nr + 1) % 3].dma_start(xc[nr][:, 4:8, :], xT[:, nr, 4:8, :])
        nc.scalar.dma_start(wo_sb, woT)
        xres_pool = ctx.enter_context(tc.tile_pool(name="xrp", bufs=2))
        xres_r = xres.rearrange("(q p) d -> q p d", p=128)
        xres_sb = [xres_pool.tile([128, D], f32, tag="xres",
                                  name=f"xres{qt}") for qt in range(4)]
        nc.scalar.dma_start(xres_sb[0], xres_r[0])
        nc.scalar.dma_start(xres_sb[1], xres_r[1])
        if has_qkv_bias:
            nc.scalar.dma_start(bqkv_sb, bqkv)
        if has_gamma:
            nc.scalar.dma_start(
                gamma_sb,
                bass.AP(tensor=gamma_d.tensor, offset=gamma_d.offset,
                        ap=[[0, 128]] + gamma_d.ap),
            )
        if has_beta:
            nc.scalar.dma_start(
                beta_sb,
                bass.AP(tensor=beta_d.tensor, offset=beta_d.offset,
                        ap=[[0, 128]] + beta_d.ap),
            )

        dram_pool = ctx.enter_context(tc.tile_pool(name="dram", bufs=1,
                                                   space="DRAM"))
        p_dram = [dram_pool.tile([NQ, D], f16, tag=f"p{qt}", name=f"p{qt}")
                  for qt in range(4)]
        pred_dram = [dram_pool.tile([128, D], f16, tag=f"pr{qt}",
                                    name=f"pr{qt}")
                     for qt in range(4)]

        # ---- PSUM pools: 2 (mm) + 4 (sps) + 2 (cps) = 8 banks ----
        mm_pool = ctx.enter_context(tc.tile_pool(name="mm", bufs=2,
                                                 space="PSUM"))
        sps_pool = ctx.enter_context(tc.tile_pool(name="sps", bufs=2,
                                                  space="PSUM"))
        cps_pool = ctx.enter_context(tc.tile_pool(name="cps", bufs=1,
                                                  space="PSUM"))
        es_pool = ctx.enter_context(tc.tile_pool(name="es", bufs=6))
        nrm_pool = ctx.enter_context(tc.tile_pool(name="nrm", bufs=2))
        psb_pool = ctx.enter_context(tc.tile_pool(name="psb", bufs=4))
        yt_pool = ctx.enter_context(tc.tile_pool(name="yt", bufs=2))

        def qkv_chunk_units(nr):
            # ~1us PE units so the feed never starves the exp stream
            ns = slice(512 * nr, 512 * (nr + 1))
            def qk_half(wsb, dst, bidx, dt_, half, ps_box):
                def unit():
                    if half == 0:
                        ps_box[0] = mm_pool.tile([128, 512], f32, tag="mm",
                                                 name=f"qk_{nr}_{bidx}_{dt_}")
                    ps = ps_box[0]
                    for ko in range(4 * half, 4 * half + 4):
                        nc.tensor.matmul(
                            ps,
                            lhsT=wsb[:, ko, 128 * dt_:128 * dt_ + 128],
                            rhs=xc[nr][:, ko, :],
                            start=(ko == 0),
                            stop=(ko == 7 and not has_qkv_bias),
                        )
                    if half == 1:
                        if has_qkv_bias:
                            nc.tensor.matmul(
                                ps,
                                lhsT=bqkv_sb[:, bidx,
                                             128 * dt_:128 * dt_ + 128],
                                rhs=ones512,
                                start=False, stop=True,
                            )
                        nc.vector.tensor_copy(out=dst[:, dt_, ns], in_=ps)
                return unit
            def v_half(vh, j2, ps_box):
                def unit():
                    if j2 == 0:
                        ps_box[0] = mm_pool.tile([128, 512], f32, tag="mm",
                                                 name=f"v_{nr}_{vh}")
                    ps = ps_box[0]
                    for ko in range(8):
                        ntl = 2 * vh + j2
                        nc.tensor.matmul(
                            ps[:, 256 * j2:256 * (j2 + 1)],
                            lhsT=xc[nr][:, ko, 128 * ntl:128 * ntl + 128],
                            rhs=wv_sb[:, ko],
                            start=(ko == 0),
                            stop=(ko == 7 and not has_qkv_bias),
                        )
                    if has_qkv_bias:
                        nc.tensor.matmul(
                            ps[:, 256 * j2:256 * (j2 + 1)],
                            lhsT=ones512[:, 0:128],
                            rhs=bqkv_sb[:, 2, :],
                            start=False, stop=True,
                        )
                    if j2 == 1:
                        nt0 = 4 * nr + 2 * vh
                        nc.vector.tensor_copy(
                            out=v_sb[:, nt0:nt0 + 2, :, 0:DH],
                            in_=ps.rearrange("p (n2 h d) -> p n2 h d",
                                             n2=2, h=HPC))
                return unit
            units = []
            for wsb, dst, bidx in ((wk_sb, kT_sb, 1), (wq_sb, qT_sb, 0)):
                for dt_ in range(2):
                    box = [None]
                    units.append(qk_half(wsb, dst, bidx, dt_, 0, box))
                    units.append(qk_half(wsb, dst, bidx, dt_, 1, box))
            for vh in range(2):
                box = [None]
                units.append(v_half(vh, 0, box))
                units.append(v_half(vh, 1, box))
            return units

        def emit_qkv_chunk(nr):
            for u in qkv_chunk_units(nr):
                u()

        def emit_recip(qt, hp, cps):
            # 1/denoms via the ~5x-faster approximate reciprocal (51 ULP is
            # plenty for softmax denominators, which are positive and O(100)).
            den_sb = nrm_pool.tile([1, 2, 512], f32, tag="den",
                                   name=f"den_{qt}_{hp}")
            nc.vector.tensor_copy(out=den_sb, in_=cps[64:65, :, :])
            rec_f = nrm_pool.tile([1, 2, 512], f32, tag="recf",
                                  name=f"recf_{qt}_{hp}")
            nc.vector.reciprocal_approx_fast(out=rec_f, in_=den_sb)
            rec_sb = nrm_pool.tile([1, 2, 512], f16, tag="rec",
                                   name=f"rec_{qt}_{hp}")
            nc.vector.tensor_copy(out=rec_sb, in_=rec_f)
            return rec_sb

        def emit_norm_tail(qt, hp, cps, rec_sb):
            for hi in range(2):
                ph = 64 * hi
                bc = mm_pool.tile([64, 512], f32, tag="mm",
                                  name=f"bc_{qt}_{hp}_{hi}")
                nc.tensor.matmul(bc, lhsT=ones64, rhs=rec_sb[:, hi, :],
                                 start=True, stop=True)
                bcs = nrm_pool.tile([64, 512], f32, tag="bcs",
                                    name=f"bcs_{qt}_{hp}_{hi}")
                nc.vector.tensor_copy(out=bcs, in_=bc)
                nc.vector.tensor_mul(
                    out=ctx_sb[ph:ph + 64, hp, 512 * qt:512 * (qt + 1)],
                    in0=cps[0:64, hi, :], in1=bcs)

        def proj_units(qt):
            # partial out-proj P_qt = ctx_qt @ WoT_local, staged to DRAM,
            # then ReduceScatter(add) over the batch's 4 cores; each core
            # receives its rank's 128 rows of this q-tile.
            def chunk(c, psb_box):
                def unit():
                    nsl = slice(512 * qt + 128 * c, 512 * qt + 128 * (c + 1))
                    p_sb = psb_pool.tile([128, D], f16, tag="psb",
                                         name=f"psb_{qt}_{c}")
                    for Dt in range(2):
                        ps = mm_pool.tile([128, 512], f32, tag="mm",
                                          name=f"pp_{qt}_{c}_{Dt}")
                        for t in range(2):
                            nc.tensor.matmul(
                                ps,
                                lhsT=ctx_sb[:, t, nsl],
                                rhs=wo_sb[:, t, 512 * Dt:512 * (Dt + 1)],
                                start=(t == 0), stop=(t == 1),
                            )
                        nc.vector.tensor_copy(
                            out=p_sb[:, 512 * Dt:512 * (Dt + 1)], in_=ps)
                    nc.sync.dma_start(p_dram[qt][128 * c:128 * (c + 1)], p_sb)
                return unit
            def rs():
                nc.gpsimd.collective_compute(
                    "ReduceScatter", ALU.add,
                    replica_groups=GROUPS,
                    ins=[p_dram[qt][:, :]],
                    outs=[pred_dram[qt][:, :]],
                )
            units = [chunk(c, None) for c in range(4)]
            units.append(rs)
            return units

        def emit_proj(qt):
            for u in proj_units(qt):
                u()

        def emit_ln(qt):
            # pinned to the end of the schedule: LN depends on the
            # ReduceScatter result, and if the scheduler hoists it into the
            # attention stream its engine queues block on the collective.
            ctx.enter_context(tc.tile_wait_until(1.0 + 0.01 * qt))
            yt16 = yt_pool.tile([128, D], f16, tag="yt16", name=f"yt16_{qt}")
            (nc.sync if qt == 3 else nc.scalar).dma_start(yt16, pred_dram[qt])
            yt = yt_pool.tile([128, D], f32, tag="yt", name=f"yt_{qt}")
            nc.vector.tensor_add(out=yt, in0=yt16, in1=xres_sb[qt])
            if qt + 2 < 4:
                nc.scalar.dma_start(xres_sb[qt + 2], xres_r[qt + 2])
            st = yt_pool.tile([128, 2, 6], f32, tag="st", name=f"st_{qt}")
            nc.vector.bn_stats(out=st[:, 0], in_=yt[:, 0:512])
            nc.vector.bn_stats(out=st[:, 1], in_=yt[:, 512:1024])
            mv = yt_pool.tile([128, 2], f32, tag="mv", name=f"mv_{qt}")
            nc.vector.bn_aggr(out=mv, in_=st)
            rstd = yt_pool.tile([128, 1], f32, tag="rstd", name=f"rstd_{qt}")
            nc.scalar.activation(out=rstd, in_=mv[:, 1:2], func=AF.Sqrt,
                                 bias=eps_sb, scale=1.0)
            nc.vector.reciprocal(out=rstd, in_=rstd)
            nc.vector.tensor_scalar(
                out=yt, in0=yt, scalar1=mv[:, 0:1], scalar2=rstd,
                op0=ALU.subtract, op1=ALU.mult)
            if has_gamma:
                nc.vector.tensor_mul(out=yt, in0=yt, in1=gamma_sb)
            if has_beta:
                nc.vector.tensor_add(out=yt, in0=yt, in1=beta_sb)
            nc.sync.dma_start(out[128 * qt:128 * (qt + 1)], yt)

        def emit_av(cps, heads, pend_item, n_kt):
            es, kt, c0 = pend_item
            for hi, h in enumerate(heads):
                nc.tensor.matmul(
                    cps[0:65, hi, c0:512],
                    lhsT=v_sb[:, kt, h, :],
                    rhs=es[:, hi, c0:512],
                    start=(kt == 0),
                    stop=(kt == n_kt - 1),
                )

        # ---------------- pipelined emission ----------------
        from collections import deque

        emit_qkv_chunk(0)
        emit_qkv_chunk(1)

        feed = deque()        # deferred ~1us PE work units, one drained per kt
        pending_norm = None   # (qt, hp, cps, rec_sb) awaiting norm tail
        for qt in range(4):
            for hp in range(2):
                if qt == 1 and hp == 0:
                    feed.extend(qkv_chunk_units(2))
                elif qt == 2 and hp == 0:
                    feed.extend(qkv_chunk_units(3))
                heads = (2 * hp, 2 * hp + 1)
                cps = cps_pool.tile([128, 2, 512], f32, tag="ctx",
                                    name=f"cps_{qt}_{hp}")
                n_kt = 4 * qt + 4
                pend = []
                for kt in range(n_kt):
                    j = kt - 4 * qt
                    c0 = 128 * j if j > 0 else 0
                    sp = sps_pool.tile([128, 2, 512], f32, tag="s",
                                       name=f"sp_{qt}_{hp}_{kt}")
                    for hi, h in enumerate(heads):
                        ph = 64 * (h % 2)
                        nc.tensor.matmul(
                            sp[:, hi, c0:512],
                            lhsT=kT_sb[ph:ph + 64, hp,
                                       128 * kt:128 * kt + 128],
                            rhs=qT_sb[ph:ph + 64, hp,
                                      512 * qt + c0:512 * (qt + 1)],
                            start=True,
                            stop=True,
                        )
                    es = es_pool.tile([128, 2, 512], f16, tag="es")
                    nc.scalar.activation(
                        out=es[:, :, c0:512],
                        in_=sp[:, :, c0:512],
                        func=AF.Exp, scale=0.125,
                    )
                    if j >= 0:
                        for hi in range(2):
                            nc.vector.tensor_mul(
                                out=es[:, hi, 128 * j:128 * j + 128],
                                in0=es[:, hi, 128 * j:128 * j + 128],
                                in1=tri_sb)
                    pend.append((es, kt, c0))
                    if len(pend) > 3:
                        emit_av(cps, heads, pend.pop(0), n_kt)
                        emit_av(cps, heads, pend.pop(0), n_kt)
                    # deferred cross-phase work rides inside the score
                    # stream so the in-order PE never drains at boundaries;
                    # one ~1us unit per kt keeps the exp stream fed
                    if kt == min(2, n_kt - 1) and pending_norm is not None:
                        nqt, nhp, _, _ = pending_norm
                        emit_norm_tail(*pending_norm)
                        pending_norm = None
                        if nhp == 1:
                            feed.extend(proj_units(nqt))
                    elif feed:
                        feed.popleft()()
                while pend:
                    emit_av(cps, heads, pend.pop(0), n_kt)
                rec_sb = emit_recip(qt, hp, cps)
                pending_norm = (qt, hp, cps, rec_sb)
        while feed:
            feed.popleft()()
        emit_norm_tail(*pending_norm)
        emit_proj(3)
        emit_ln(0)
        emit_ln(1)
        emit_ln(2)
        emit_ln(3)

    nc.compile()
    return nc


def build_nc(flags=(False, False, False)):
    if flags not in _CACHE:
        _CACHE[flags] = _build(flags)
    return _CACHE[flags]


def make_in_maps(inputs):
    x = np.ascontiguousarray(np.asarray(inputs["x"], dtype=np.float32))
    Wq = np.asarray(inputs["Wq"], np.float32)
    Wk = np.asarray(inputs["Wk"], np.float32)
    Wv = np.asarray(inputs["Wv"], np.float32)
    Wo = np.asarray(inputs["Wo"], np.float32)
    bq = np.asarray(inputs["bq"], np.float32)
    bk = np.asarray(inputs["bk"], np.float32)
    bv = np.asarray(inputs["bv"], np.float32)
    bo = np.asarray(inputs["bo"], np.float32)
    gamma = np.asarray(inputs["ln_gamma"], np.float32)
    beta = np.asarray(inputs["ln_beta"], np.float32)

    has_qkv_bias = bool(np.any(bq) or np.any(bk) or np.any(bv))
    has_gamma = not np.allclose(gamma, 1.0)
    has_beta = bool(np.any(beta))
    flags = (has_qkv_bias, has_gamma, has_beta)

    xres_full = x + bo  # residual with output bias folded in

    in_maps = []
    for c in range(NCORES):
        b, r = c // 4, c % 4
        cols = slice(DP * r, DP * (r + 1))
        xres_rows = np.concatenate(
            [xres_full[b, 512 * qt + 128 * r:512 * qt + 128 * (r + 1)]
             for qt in range(4)], axis=0)
        # striped device layouts: [p, ko, m] for weights (contraction index
        # k = ko*128 + p), [p, nr, ko, n] for x^T, [p, t, o] for Wo cols.
        xTb = x[b].T.reshape(8, 128, 4, 512).transpose(1, 2, 0, 3)
        m = {
            "xT": np.ascontiguousarray(xTb.astype(F16)),
            "xres": np.ascontiguousarray(xres_rows),
            "wqT": np.ascontiguousarray(
                Wq[cols, :].T.reshape(8, 128, DP).transpose(1, 0, 2).astype(F16)),
            "wkT": np.ascontiguousarray(
                Wk[cols, :].T.reshape(8, 128, DP).transpose(1, 0, 2).astype(F16)),
            "wvT": np.ascontiguousarray(
                Wv[cols, :].T.reshape(8, 128, DP).transpose(1, 0, 2).astype(F16)),
            "woT": np.ascontiguousarray(
                Wo[:, cols].T.reshape(2, 128, D).transpose(1, 0, 2).astype(F16)),
        }
        if has_qkv_bias:
            m["bqkv"] = np.ascontiguousarray(
                np.stack([bq[cols], bk[cols], bv[cols]])[None])
        if has_gamma:
            m["gamma"] = gamma
        if has_beta:
            m["beta"] = beta
        in_maps.append(m)
    return flags, in_maps


def assemble(results):
    """results: list of per-core dicts with 'out' [512, 1024] (4 q-tile
    chunks of 128 rows each)."""
    full = np.empty((B, N, D), dtype=np.float32)
    for c in range(NCORES):
        b, r = c // 4, c % 4
        o = results[c]["out"]
        for qt in range(4):
            full[b, 512 * qt + 128 * r:512 * qt + 128 * (r + 1)] = \
                o[128 * qt:128 * (qt + 1)]
    return full


def kernel(**inputs):
    from concourse.bass_utils import run_bass_kernel_spmd

    flags, in_maps = make_in_maps(inputs)
    nc = build_nc(flags)
    res = run_bass_kernel_spmd(nc, in_maps, core_ids=list(range(NCORES)))
    return assemble(res.results)


# revision 23
# speedup vs baseline: 1.0014x; 1.0014x over previous
"""Fused causal-attention block (QKV proj + causal softmax attention + out proj
+ residual + LayerNorm) on 8 Trainium2 NeuronCores.

Sharding: core c -> batch b = c//4, head-group r = c%4 (heads 4r..4r+3,
d' columns 256r..256r+256).  Single pipelined stream per core:

  for n-chunk nr (512 rows):   K/Q/V projections for the head group
  interleaved per q-tile qt:   flash-style causal attention (no max
                               subtraction -- scores are O(1)),
  then per q-tile:             partial output projection
                               P_qt = ctx_qt @ Wo[:, cols]^T  [512, 1024]
                               ReduceScatter(add) over the batch's 4 cores
                               -> this core's 128 rows of q-tile qt,
  then per received chunk:     + residual, LayerNorm, store.

QKV chunks, attention pairs, partial projections and collectives are
interleaved in emission order so the PE never drains between phases; the
trailing exposed work after the last attention tile is just one partial
projection + ReduceScatter + LayerNorm on 128 rows.

Core (b, r) returns out rows {512*qt + 128*r .. +128} for qt in 0..3 (the
ReduceScatter hands rank r the r-th 128-row slice of each 512-row q-tile);
the host reassembles.

All matmuls run as float32r (full-rate fp32 on the PE).  The causal mask on
diagonal 128x128 blocks is applied by multiplying the exponentiated scores
with an upper-triangular 0/1 matrix.  Softmax denominators come from an
all-ones column appended to V; their reciprocals are computed with the 512
denominators spread over 64 partitions via SBUF->SBUF DMA (partition-parallel
reciprocal), then broadcast with a ones-column matmul.  The two heads of a
partition-tile pair compute their scores back-to-back at PE base partitions
0/64 into one shared [128, 2, 512] PSUM tile that a single strided ACT call
exponentiates for both heads.
"""

import numpy as np
F16 = np.float16
B, N, D = 2, 2048, 1024
H, DH = 16, 64
NCORES = 8
HPC = 4          # heads per core
DP = HPC * DH    # 256 d' columns per core
NQ = N // 4      # 512 rows per q-tile
LN_EPS = 1e-5
GROUPS = [[0, 1, 2, 3], [4, 5, 6, 7]]

_CACHE = {}


def _build(flags):
    """Build+compile the Bacc program. flags = (has_qkv_bias, has_gamma, has_beta)."""
    import concourse.bass as bass
    import concourse.bacc as bacc
    import concourse.tile as tile
    from concourse import mybir
    from contextlib import ExitStack

    has_qkv_bias, has_gamma, has_beta = flags
    f32 = mybir.dt.float32
    f32r = mybir.dt.float32r
    f16 = mybir.dt.float16
    bf16 = mybir.dt.bfloat16
    AF = mybir.ActivationFunctionType
    ALU = mybir.AluOpType

    nc = bacc.Bacc(
        trn_type="TRN2",
        target_bir_lowering=False,
        debug=False,
        num_devices=NCORES,
    )

    xT = nc.dram_tensor("xT", [128, 4, 8, 512], f16, kind="ExternalInput").ap()
    xres = nc.dram_tensor("xres", [NQ, D], f32, kind="ExternalInput").ap()
    wqT = nc.dram_tensor("wqT", [128, 8, DP], f16, kind="ExternalInput").ap()
    wkT = nc.dram_tensor("wkT", [128, 8, DP], f16, kind="ExternalInput").ap()
    wvT = nc.dram_tensor("wvT", [128, 8, DP], f16, kind="ExternalInput").ap()
    woT = nc.dram_tensor("woT", [128, 2, D], f16, kind="ExternalInput").ap()
    out = nc.dram_tensor("out", [NQ, D], f32, kind="ExternalOutput").ap()
    if has_qkv_bias:
        bqkv = nc.dram_tensor("bqkv", [1, 3, DP], f32r, kind="ExternalInput").ap()
    if has_gamma:
        gamma_d = nc.dram_tensor("gamma", [D], f32, kind="ExternalInput").ap()
    if has_beta:
        beta_d = nc.dram_tensor("beta", [D], f32, kind="ExternalInput").ap()

    # multiplicative causal mask for diagonal blocks: keep k <= q
    # (partition p = k offset, free c = q offset)
    tri_np = np.triu(np.ones((128, 128), np.float32))
    tri_d = nc.inline_tensor(np.ascontiguousarray(tri_np.astype(np.float16)),
                             name="tri_const").ap()

    with tile.TileContext(nc) as tc, ExitStack() as ctx, \
            nc.allow_low_precision(reason="float32r carries full fp32 bits"):
        singles = ctx.enter_context(tc.tile_pool(name="singles", bufs=1))
        qkv_pool = ctx.enter_context(tc.tile_pool(name="qkv", bufs=1))

        # weights, striped k-on-partitions
        wq_sb = singles.tile([128, 8, DP], f16, tag="wq")
        wk_sb = singles.tile([128, 8, DP], f16, tag="wk")
        wv_sb = singles.tile([128, 8, DP], f16, tag="wv")
        wo_sb = singles.tile([128, 2, D], f16, tag="wo")

        tri_sb = singles.tile([128, 128], f16, tag="tri")
        ones_f32 = singles.tile([128, 64], f32, tag="ones_f32")
        nc.vector.memset(ones_f32, 1.0)
        ones64 = singles.tile([1, 64], f16, tag="ones64")
        nc.vector.tensor_copy(out=ones64, in_=ones_f32[0:1, :])
        eps_sb = singles.tile([128, 1], f32, tag="eps")
        nc.vector.memset(eps_sb, LN_EPS)
        if has_qkv_bias:
            o512f = singles.tile([1, 512], f32, tag="o512f")
            nc.vector.memset(o512f, 1.0)
            ones512 = singles.tile([1, 512], f32r, tag="ones512")
            nc.vector.tensor_copy(out=ones512, in_=o512f)
            bqkv_sb = singles.tile([1, 3, DP], f32r, tag="bqkv")
        if has_gamma:
            gamma_sb = singles.tile([128, D], f32, tag="gamma")
        if has_beta:
            beta_sb = singles.tile([128, D], f32, tag="beta")

        # persistent activations
        qT_sb = qkv_pool.tile([128, 2, N], f16, tag="qT")   # Q^T [d'(256), n]
        kT_sb = qkv_pool.tile([128, 2, N], f16, tag="kT")   # K^T [d'(256), n]
        v_sb = qkv_pool.tile([128, 16, HPC, DH + 1], f16, tag="v")  # V + ones
        ctx_sb = qkv_pool.tile([128, 2, N], f16, tag="ctxT")  # normalized ctx^T
        nc.vector.tensor_copy(
            out=v_sb[:, :, :, DH:DH + 1],
            in_=ones_f32.rearrange("p (a b c) -> p a b c", a=16, b=4))


        # ---- input DMAs (host-prearranged contiguous layouts) ----
        xt_pool = ctx.enter_context(tc.tile_pool(name="xt", bufs=4))
        xc = []
        for nr in range(4):
            xc.append(xt_pool.tile([128, 8, 512], f16, tag="xc",
                                   name=f"xc{nr}"))
        # first chunk: per-ko pieces round-robin over three queues so the
        # first K accumulation chain can start as pieces land; later chunks
        # as halves on sync/gpsimd.
        nc.sync.dma_start(wk_sb, wkT)
        nc.scalar.dma_start(tri_sb, tri_d)
        dq = [nc.sync, nc.gpsimd, nc.scalar]
        for ko in range(8):
            dq[ko % 3].dma_start(xc[0][:, ko, :], xT[:, 0, ko, :])
        nc.scalar.dma_start(wq_sb, wqT)
        nc.gpsimd.dma_start(wv_sb, wvT)
        dq2 = [nc.sync, nc.gpsimd, nc.scalar]
        for nr in range(1, 4):
            dq2[(2 * nr) % 3].dma_start(xc[nr][:, 0:4, :], xT[:, nr, 0:4, :])
            dq2[(2 * nr + 1) % 3].dma_start(xc[nr][:, 4:8, :], xT[:, nr, 4:8, :])
        nc.scalar.dma_start(wo_sb, woT)
        xres_pool = ctx.enter_context(tc.tile_pool(name="xrp", bufs=2))
        xres_r = xres.rearrange("(q p) d -> q p d", p=128)
        xres_sb = [xres_pool.tile([128, D], f32, tag="xres",
                                  name=f"xres{qt}") for qt in range(4)]
        nc.scalar.dma_start(xres_sb[0], xres_r[0])
        nc.scalar.dma_start(xres_sb[1], xres_r[1])
        if has_qkv_bias:
            nc.scalar.dma_start(bqkv_sb, bqkv)
        if has_gamma:
            nc.scalar.dma_start(
                gamma_sb,
                bass.AP(tensor=gamma_d.tensor, offset=gamma_d.offset,
                        ap=[[0, 128]] + gamma_d.ap),
            )
        if has_beta:
            nc.scalar.dma_start(
                beta_sb,
                bass.AP(tensor=beta_d.tensor, offset=beta_d.offset,
                        ap=[[0, 128]] + beta_d.ap),
            )

        dram_pool = ctx.enter_context(tc.tile_pool(name="dram", bufs=1,
                                                   space="DRAM"))
        p_dram = [dram_pool.tile([NQ, D], f16, tag=f"p{qt}", name=f"p{qt}")
                  for qt in range(4)]
        pred_dram = [dram_pool.tile([128, D], f16, tag=f"pr{qt}",
                                    name=f"pr{qt}")
                     for qt in range(4)]

        # ---- PSUM pools: 2 (mm) + 4 (sps) + 2 (cps) = 8 banks ----
        mm_pool = ctx.enter_context(tc.tile_pool(name="mm", bufs=2,
                                                 space="PSUM"))
        sps_pool = ctx.enter_context(tc.tile_pool(name="sps", bufs=2,
                                                  space="PSUM"))
        cps_pool = ctx.enter_context(tc.tile_pool(name="cps", bufs=1,
                                                  space="PSUM"))
        es_pool = ctx.enter_context(tc.tile_pool(name="es", bufs=6))
        nrm_pool = ctx.enter_context(tc.tile_pool(name="nrm", bufs=2))
        psb_pool = ctx.enter_context(tc.tile_pool(name="psb", bufs=4))
        yt_pool = ctx.enter_context(tc.tile_pool(name="yt", bufs=2))

        def qkv_chunk_units(nr):
            # ~1us PE units so the feed never starves the exp stream
            ns = slice(512 * nr, 512 * (nr + 1))
            def qk_half(wsb, dst, bidx, dt_, half, ps_box):
                def unit():
                    if half == 0:
                        ps_box[0] = mm_pool.tile([128, 512], f32, tag="mm",
                                                 name=f"qk_{nr}_{bidx}_{dt_}")
                    ps = ps_box[0]
                    for ko in range(4 * half, 4 * half + 4):
                        nc.tensor.matmul(
                            ps,
                            lhsT=wsb[:, ko, 128 * dt_:128 * dt_ + 128],
                            rhs=xc[nr][:, ko, :],
                            start=(ko == 0),
                            stop=(ko == 7 and not has_qkv_bias),
                        )
                    if half == 1:
                        if has_qkv_bias:
                            nc.tensor.matmul(
                                ps,
                                lhsT=bqkv_sb[:, bidx,
                                             128 * dt_:128 * dt_ + 128],
                                rhs=ones512,
                                start=False, stop=True,
                            )
                        nc.vector.tensor_copy(out=dst[:, dt_, ns], in_=ps)
                return unit
            def v_half(vh, j2, ps_box):
                def unit():
                    if j2 == 0:
                        ps_box[0] = mm_pool.tile([128, 512], f32, tag="mm",
                                                 name=f"v_{nr}_{vh}")
                    ps = ps_box[0]
                    for ko in range(8):
                        ntl = 2 * vh + j2
                        nc.tensor.matmul(
                            ps[:, 256 * j2:256 * (j2 + 1)],
                            lhsT=xc[nr][:, ko, 128 * ntl:128 * ntl + 128],
                            rhs=wv_sb[:, ko],
                            start=(ko == 0),
                            stop=(ko == 7 and not has_qkv_bias),
                        )
                    if has_qkv_bias:
                        nc.tensor.matmul(
                            ps[:, 256 * j2:256 * (j2 + 1)],
                            lhsT=ones512[:, 0:128],
                            rhs=bqkv_sb[:, 2, :],
                            start=False, stop=True,
                        )
                    if j2 == 1:
                        nt0 = 4 * nr + 2 * vh
                        nc.vector.tensor_copy(
                            out=v_sb[:, nt0:nt0 + 2, :, 0:DH],
                            in_=ps.rearrange("p (n2 h d) -> p n2 h d",
                                             n2=2, h=HPC))
                return unit
            units = []
            for wsb, dst, bidx in ((wk_sb, kT_sb, 1), (wq_sb, qT_sb, 0)):
                for dt_ in range(2):
                    box = [None]
                    units.append(qk_half(wsb, dst, bidx, dt_, 0, box))
                    units.append(qk_half(wsb, dst, bidx, dt_, 1, box))
            for vh in range(2):
                box = [None]
                units.append(v_half(vh, 0, box))
                units.append(v_half(vh, 1, box))
            return units

        def emit_qkv_chunk(nr):
            for u in qkv_chunk_units(nr):
                u()

        def emit_recip(qt, hp, cps):
            # 1/denoms via the ~5x-faster approximate reciprocal (51 ULP is
            # plenty for softmax denominators, which are positive and O(100)).
            den_sb = nrm_pool.tile([1, 2, 512], f32, tag="den",
                                   name=f"den_{qt}_{hp}")
            nc.vector.tensor_copy(out=den_sb, in_=cps[64:65, :, :])
            rec_f = nrm_pool.tile([1, 2, 512], f32, tag="recf",
                                  name=f"recf_{qt}_{hp}")
            nc.vector.reciprocal_approx_fast(out=rec_f, in_=den_sb)
            rec_sb = nrm_pool.tile([1, 2, 512], f16, tag="rec",
                                   name=f"rec_{qt}_{hp}")
            nc.vector.tensor_copy(out=rec_sb, in_=rec_f)
            return rec_sb

        def emit_norm_tail(qt, hp, cps, rec_sb):
            for hi in range(2):
                ph = 64 * hi
                bc = mm_pool.tile([64, 512], f32, tag="mm",
                                  name=f"bc_{qt}_{hp}_{hi}")
                nc.tensor.matmul(bc, lhsT=ones64, rhs=rec_sb[:, hi, :],
                                 start=True, stop=True)
                bcs = nrm_pool.tile([64, 512], f32, tag="bcs",
                                    name=f"bcs_{qt}_{hp}_{hi}")
                nc.vector.tensor_copy(out=bcs, in_=bc)
                nc.vector.tensor_mul(
                    out=ctx_sb[ph:ph + 64, hp, 512 * qt:512 * (qt + 1)],
                    in0=cps[0:64, hi, :], in1=bcs)

        def proj_units(qt):
            # partial out-proj P_qt = ctx_qt @ WoT_local, staged to DRAM,
            # then ReduceScatter(add) over the batch's 4 cores; each core
            # receives its rank's 128 rows of this q-tile.
            def chunk(c, psb_box):
                def unit():
                    nsl = slice(512 * qt + 128 * c, 512 * qt + 128 * (c + 1))
                    p_sb = psb_pool.tile([128, D], f16, tag="psb",
                                         name=f"psb_{qt}_{c}")
                    for Dt in range(2):
                        ps = mm_pool.tile([128, 512], f32, tag="mm",
                                          name=f"pp_{qt}_{c}_{Dt}")
                        for t in range(2):
                            nc.tensor.matmul(
                                ps,
                                lhsT=ctx_sb[:, t, nsl],
                                rhs=wo_sb[:, t, 512 * Dt:512 * (Dt + 1)],
                                start=(t == 0), stop=(t == 1),
                            )
                        nc.vector.tensor_copy(
                            out=p_sb[:, 512 * Dt:512 * (Dt + 1)], in_=ps)
                    nc.sync.dma_start(p_dram[qt][128 * c:128 * (c + 1)], p_sb)
                return unit
            def rs():
                nc.gpsimd.collective_compute(
                    "ReduceScatter", ALU.add,
                    replica_groups=GROUPS,
                    ins=[p_dram[qt][:, :]],
                    outs=[pred_dram[qt][:, :]],
                )
            units = [chunk(c, None) for c in range(4)]
            units.append(rs)
            return units

        def emit_proj(qt):
            for u in proj_units(qt):
                u()

        def emit_ln(qt):
            # pinned to the end of the schedule: LN depends on the
            # ReduceScatter result, and if the scheduler hoists it into the
            # attention stream its engine queues block on the collective.
            ctx.enter_context(tc.tile_wait_until(1.0 + 0.01 * qt))
            yt16 = yt_pool.tile([128, D], f16, tag="yt16", name=f"yt16_{qt}")
            (nc.gpsimd if qt == 3 else nc.scalar).dma_start(yt16, pred_dram[qt])
            yt = yt_pool.tile([128, D], f32, tag="yt", name=f"yt_{qt}")
            nc.vector.tensor_add(out=yt, in0=yt16, in1=xres_sb[qt])
            if qt + 2 < 4:
                nc.scalar.dma_start(xres_sb[qt + 2], xres_r[qt + 2])
            st = yt_pool.tile([128, 2, 6], f32, tag="st", name=f"st_{qt}")
            nc.vector.bn_stats(out=st[:, 0], in_=yt[:, 0:512])
            nc.vector.bn_stats(out=st[:, 1], in_=yt[:, 512:1024])
            mv = yt_pool.tile([128, 2], f32, tag="mv", name=f"mv_{qt}")
            nc.vector.bn_aggr(out=mv, in_=st)
            rstd = yt_pool.tile([128, 1], f32, tag="rstd", name=f"rstd_{qt}")
            nc.scalar.activation(out=rstd, in_=mv[:, 1:2], func=AF.Sqrt,
                                 bias=eps_sb, scale=1.0)
            nc.vector.reciprocal(out=rstd, in_=rstd)
            nc.vector.tensor_scalar(
                out=yt, in0=yt, scalar1=mv[:, 0:1], scalar2=rstd,
                op0=ALU.subtract, op1=ALU.mult)
            if has_gamma:
                nc.vector.tensor_mul(out=yt, in0=yt, in1=gamma_sb)
            if has_beta:
                nc.vector.tensor_add(out=yt, in0=yt, in1=beta_sb)
            nc.sync.dma_start(out[128 * qt:128 * (qt + 1)], yt)

        def emit_av(cps, heads, pend_item, n_kt):
            es, kt, c0 = pend_item
            for hi, h in enumerate(heads):
                nc.tensor.matmul(
                    cps[0:65, hi, c0:512],
                    lhsT=v_sb[:, kt, h, :],
                    rhs=es[:, hi, c0:512],
                    start=(kt == 0),
                    stop=(kt == n_kt - 1),
                )

        # ---------------- pipelined emission ----------------
        from collections import deque

        emit_qkv_chunk(0)

        feed = deque()        # deferred ~1us PE work units, one drained per kt
        feed.extend(qkv_chunk_units(1))
        pending_norm = None   # (qt, hp, cps, rec_sb) awaiting norm tail
        for qt in range(4):
            for hp in range(2):
                if qt == 1 and hp == 0:
                    feed.extend(qkv_chunk_units(2))
                elif qt == 2 and hp == 0:
                    feed.extend(qkv_chunk_units(3))
                heads = (2 * hp, 2 * hp + 1)
                cps = cps_pool.tile([128, 2, 512], f32, tag="ctx",
                                    name=f"cps_{qt}_{hp}")
                n_kt = 4 * qt + 4
                pend = []
                for kt in range(n_kt):
                    j = kt - 4 * qt
                    c0 = 128 * j if j > 0 else 0
                    sp = sps_pool.tile([128, 2, 512], f32, tag="s",
                                       name=f"sp_{qt}_{hp}_{kt}")
                    for hi, h in enumerate(heads):
                        ph = 64 * (h % 2)
                        nc.tensor.matmul(
                            sp[:, hi, c0:512],
                            lhsT=kT_sb[ph:ph + 64, hp,
                                       128 * kt:128 * kt + 128],
                            rhs=qT_sb[ph:ph + 64, hp,
                                      512 * qt + c0:512 * (qt + 1)],
                            start=True,
                            stop=True,
                        )
                    es = es_pool.tile([128, 2, 512], f16, tag="es")
                    nc.scalar.activation(
                        out=es[:, :, c0:512],
                        in_=sp[:, :, c0:512],
                        func=AF.Exp, scale=0.125,
                    )
                    if j >= 0:
                        for hi in range(2):
                            nc.vector.tensor_mul(
                                out=es[:, hi, 128 * j:128 * j + 128],
                                in0=es[:, hi, 128 * j:128 * j + 128],
                                in1=tri_sb)
                    pend.append((es, kt, c0))
                    if len(pend) > 3:
                        emit_av(cps, heads, pend.pop(0), n_kt)
                        emit_av(cps, heads, pend.pop(0), n_kt)
                    # deferred cross-phase work rides inside the score
                    # stream so the in-order PE never drains at boundaries;
                    # one ~1us unit per kt keeps the exp stream fed
                    if kt == min(2, n_kt - 1) and pending_norm is not None:
                        nqt, nhp, _, _ = pending_norm
                        emit_norm_tail(*pending_norm)
                        pending_norm = None
                        if nhp == 1:
                            feed.extend(proj_units(nqt))
                    elif feed:
                        feed.popleft()()
                while pend:
                    emit_av(cps, heads, pend.pop(0), n_kt)
                rec_sb = emit_recip(qt, hp, cps)
                pending_norm = (qt, hp, cps, rec_sb)
        while feed:
            feed.popleft()()
        emit_norm_tail(*pending_norm)
        emit_proj(3)
        emit_ln(0)
        emit_ln(1)
        emit_ln(2)
        emit_ln(3)

    nc.compile()
    return nc


def build_nc(flags=(False, False, False)):
    if flags not in _CACHE:
        _CACHE[flags] = _build(flags)
    return _CACHE[flags]


def make_in_maps(inputs):
    x = np.ascontiguousarray(np.asarray(inputs["x"], dtype=np.float32))
    Wq = np.asarray(inputs["Wq"], np.float32)
    Wk = np.asarray(inputs["Wk"], np.float32)
    Wv = np.asarray(inputs["Wv"], np.float32)
    Wo = np.asarray(inputs["Wo"], np.float32)
    bq = np.asarray(inputs["bq"], np.float32)
    bk = np.asarray(inputs["bk"], np.float32)
    bv = np.asarray(inputs["bv"], np.float32)
    bo = np.asarray(inputs["bo"], np.float32)
    gamma = np.asarray(inputs["ln_gamma"], np.float32)
    beta = np.asarray(inputs["ln_beta"], np.float32)

    has_qkv_bias = bool(np.any(bq) or np.any(bk) or np.any(bv))
    has_gamma = not np.allclose(gamma, 1.0)
    has_beta = bool(np.any(beta))
    flags = (has_qkv_bias, has_gamma, has_beta)

    xres_full = x + bo  # residual with output bias folded in

    in_maps = []
    for c in range(NCORES):
        b, r = c // 4, c % 4
        cols = slice(DP * r, DP * (r + 1))
        xres_rows = np.concatenate(
            [xres_full[b, 512 * qt + 128 * r:512 * qt + 128 * (r + 1)]
             for qt in range(4)], axis=0)
        # striped device layouts: [p, ko, m] for weights (contraction index
        # k = ko*128 + p), [p, nr, ko, n] for x^T, [p, t, o] for Wo cols.
        xTb = x[b].T.reshape(8, 128, 4, 512).transpose(1, 2, 0, 3)
        m = {
            "xT": np.ascontiguousarray(xTb.astype(F16)),
            "xres": np.ascontiguousarray(xres_rows),
            "wqT": np.ascontiguousarray(
                Wq[cols, :].T.reshape(8, 128, DP).transpose(1, 0, 2).astype(F16)),
            "wkT": np.ascontiguousarray(
                Wk[cols, :].T.reshape(8, 128, DP).transpose(1, 0, 2).astype(F16)),
            "wvT": np.ascontiguousarray(
                Wv[cols, :].T.reshape(8, 128, DP).transpose(1, 0, 2).astype(F16)),
            "woT": np.ascontiguousarray(
                Wo[:, cols].T.reshape(2, 128, D).transpose(1, 0, 2).astype(F16)),
        }
        if has_qkv_bias:
            m["bqkv"] = np.ascontiguousarray(
                np.stack([bq[cols], bk[cols], bv[cols]])[None])
        if has_gamma:
            m["gamma"] = gamma
        if has_beta:
            m["beta"] = beta
        in_maps.append(m)
    return flags, in_maps


def assemble(results):
    """results: list of per-core dicts with 'out' [512, 1024] (4 q-tile
    chunks of 128 rows each)."""
    full = np.empty((B, N, D), dtype=np.float32)
    for c in range(NCORES):
        b, r = c // 4, c % 4
        o = results[c]["out"]
        for qt in range(4):
            full[b, 512 * qt + 128 * r:512 * qt + 128 * (r + 1)] = \
                o[128 * qt:128 * (qt + 1)]
    return full


def kernel(**inputs):
    from concourse.bass_utils import run_bass_kernel_spmd

    flags, in_maps = make_in_maps(inputs)
    nc = build_nc(flags)
    res = run_bass_kernel_spmd(nc, in_maps, core_ids=list(range(NCORES)))
    return assemble(res.results)
